# revision 24
# baseline (speedup 1.0000x reference)
"""Trainium2 Bass kernel for ContinuousAttention (self-keyed RoPE attention,
strictly-causal masked scores, no softmax).

Reference computation (B=2, NH=16, T=2048, N=256, fp32):
    QR = rope(Q)                      # interleaved-pair RoPE, freqs quantized in pairs
    S  = QR @ QR^T                    # per (b, h); K input is unused by the module
    O  = (S * strict_causal_mask) @ V

Sharding: 32 (b*nh) heads over 8 NeuronCores, 4 heads per core; no
communication.  Each core runs an identical program on its head slice.

v3 design (fp16 operands, fp32 PSUM accumulation, wide moving operands):
  - RoPE is elementwise preprocessing (O(T*N) vs the O(T^2*N) matmuls) and is
    folded into the host-side input marshalling: the host ships QR^T directly
    in fp16 (n, t) layout, plus V in fp16.  No tables, no on-device RoPE.
  - matmul1 computes transposed-score strips S^T[j-block, t-cols] in 512-wide
    column groups G: j-blocks strictly below the diagonal get N=512 moving
    columns (LDWEIGHTS fully hidden behind 2x212ns of compute); the two
    diagonal-adjacent j-blocks of each group get narrow (256/128) windows so
    no masked-out upper-triangle work is done.  PSUM->SBUF copies cast to
    fp16 and multiply the strict-causal mask on diagonal blocks.
  - matmul2 uses V as the stationary operand and the S^T strips as wide
    moving operands, accumulating O^T[n-chunk, i-cols] in PSUM over all
    j <= i: per 512-col group G and n-chunk c it is one accumulation chain
    of 4G+4 matmuls.  80 matmuls/head instead of 136.
  - Output is written as O^T (h, n, t) in fp16; the host transposes back.
"""

import math
import sys

import numpy as np

if "/opt/trn_rl_repo" not in sys.path:
    sys.path.insert(0, "/opt/trn_rl_repo")

import concourse.bass as bass
import concourse.mybir as mybir
import concourse.tile as tile
from concourse.bass_utils import run_bass_kernel_spmd

B, NH, T, N = 2, 16, 2048, 256
THETA = 2 ** 16
N_CORES = 8
H_PER_CORE = (B * NH) // N_CORES

F32 = mybir.dt.float32
FP16 = mybir.dt.float16
MULT = mybir.AluOpType.mult
HF = np.float16


def _split_overloaded_waits(nc, max_waits=1):
    """walrus in this container rejects >1 sync-wait per instruction; move
    extra waits onto preceding same-engine NoOps (semantically identical)."""
    n_split = 0
    for f in nc.m.functions:
        for bb in f.blocks:
            new_list = []
            changed = False
            for ins in bb.instructions:
                si = getattr(ins, "sync_info", None)
                if si is not None and len(si.on_wait) > max_waits:
                    waits = list(si.on_wait)
                    extra, keep = waits[:-max_waits], waits[-max_waits:]
                    k = 0
                    while extra:
                        chunk, extra = extra[:max_waits], extra[max_waits:]
                        nop = mybir.InstNoOp(
                            name=f"{ins.name}_wsplit{k}", ins=[], outs=[]
                        )
                        nop.engine = ins.engine
                        nop.sync_info = mybir.SyncInfo(on_wait=chunk, on_update=[])
                        new_list.append(nop)
                        k += 1
                    ins.sync_info = mybir.SyncInfo(
                        on_wait=keep, on_update=list(si.on_update)
                    )
                    changed = True
                    n_split += 1
                new_list.append(ins)
            if changed:
                bb.instructions = new_list
    return n_split


def rope_tables(t=T, n=N, dtype=np.float64):
    """cos table and sign-folded sin table, natural (t, n) layout."""
    idx = np.floor(np.arange(n, dtype=dtype) / dtype(2.0)) * dtype(2.0)
    freqs = (
        dtype(1.0) / (dtype(THETA) ** (idx / dtype(n))) / dtype(2.0 * math.pi)
    ).astype(dtype)
    phases = np.arange(t, dtype=dtype)[:, None] * freqs[None, :]
    ph = (phases % dtype(1.0)) * dtype(2.0 * math.pi)
    cos = np.cos(ph).astype(dtype)
    sin = np.sin(ph).astype(dtype)
    sin_a = sin.copy()
    sin_a[:, 0::2] *= dtype(-1.0)  # fold the rotate-pair sign into sin
    return cos, sin_a


def build_nc(h_per_core=H_PER_CORE, t=T, n=N, waitsplit=True):
    assert n == 256 and t % 512 == 0
    nt = t // 128  # 128-row t-tiles (16)
    ngr = t // 512  # 512-wide column groups (4)
    nc = bass.Bass("TRN2", target_bir_lowering=False, debug=False)

    qrt = nc.dram_tensor("qrt", [h_per_core, n, t], FP16, kind="ExternalInput").ap()
    # v is shipped pre-arranged in the SBUF layout [p, t-tile, n] so each
    # partition line is one fat contiguous 8KB DMA stream (vs 512B packets
    # for the strided gather from natural [t, n] layout)
    v = nc.dram_tensor(
        "v", [h_per_core, 128, (t // 128) * n], FP16, kind="ExternalInput"
    ).ap()
    o = nc.dram_tensor("o", [h_per_core, n, t], FP16, kind="ExternalOutput").ap()

    with tile.TileContext(nc) as tc:
        with (
            tc.tile_pool(name="const", bufs=1) as cpool,
            tc.tile_pool(name="q", bufs=2) as qpool,
            tc.tile_pool(name="vh", bufs=2) as vpool,
            tc.tile_pool(name="strips", bufs=2) as strippool,
            tc.tile_pool(name="ot", bufs=2) as otpool,
            tc.tile_pool(name="sps", bufs=4, space="PSUM") as spool,
            tc.tile_pool(name="ops", bufs=1, space="PSUM") as opool,
        ):
            # strict-causal mask, (s, t) orientation: strict-lower mask in the
            # first 128 cols (keep iff free > part), ones after.  Every
            # diagonal-block window starts at its diagonal, so mask512[:, :w]
            # is the right mask for any window width w in {128,256,384,512}.
            mask512 = cpool.tile([128, 512], F32)
            nc.gpsimd.memset(mask512, 1.0)
            nc.gpsimd.affine_select(
                out=mask512[:, 0:128],
                in_=mask512[:, 0:128],
                compare_op=mybir.AluOpType.is_ge,
                fill=0.0,
                base=-1,
                pattern=[[1, 128]],
                channel_multiplier=-1,
            )

            cp = 0  # copy-engine round robin

            def xcopy(dst, src):
                # gpsimd cannot read PSUM on TRN2, so only DVE + Act rotate
                nonlocal cp
                if cp % 2 == 0:
                    nc.vector.tensor_copy(out=dst, in_=src)
                else:
                    nc.scalar.copy(out=dst, in_=src)
                cp += 1

            def mcopy(dst, src, m):
                # masked (diagonal) copies need tensor_tensor -> DVE only
                nc.vector.tensor_tensor(out=dst, in0=src, in1=m, op=MULT)

            def emit_input_dmas(h):
                # all inputs on the sync queue, issued in the order compute
                # consumes them: qrt cols [0:512) (first mm1 group), cols
                # [512:1024), V rows [0:512), the rest of qrt, rest of V
                qc = [
                    qpool.tile([128, t], FP16, tag=f"q{c}", name=f"q{c}")
                    for c in range(2)
                ]
                vh = vpool.tile([128, nt * n], FP16, tag="vh", name="vh")

                def qseg(lo, hi):
                    for c in range(2):
                        nc.sync.dma_start(
                            out=qc[c][:, lo:hi],
                            in_=qrt[h][c * 128:(c + 1) * 128, lo:hi],
                        )

                def vseg(G):
                    # scalar queue: V streams in parallel with qrt on sync
                    sl = slice(4 * G * n, 4 * (G + 1) * n)
                    nc.scalar.dma_start(out=vh[:, sl], in_=v[h][:, sl])

                qseg(0, 512)
                qseg(512, 1024)
                vseg(0)
                qseg(1024, 2048)
                vseg(1)
                vseg(2)
                vseg(3)
                return qc, vh

            def alloc_head(h):
                qc, vh = emit_input_dmas(h)
                # per-head strips: S^T[j-block, t-cols >= 256*(j//2)] fp16
                strips = [
                    strippool.tile(
                        [128, t - 256 * (j // 2)], FP16,
                        tag=f"strip{j}", name=f"strip{j}",
                    )
                    for j in range(nt)
                ]
                # O^T staging, fp16, per n-chunk
                ot_sb = [
                    otpool.tile([128, t], FP16, tag=f"ot{c}", name=f"ot{c}")
                    for c in range(2)
                ]
                return {"h": h, "qc": qc, "vh": vh, "strips": strips,
                        "ot": ot_sb}

            def mm1_chains(st, specs):
                # interleave the accumulation chains of several j-blocks
                # (each in its own PSUM tile/bank) so consecutive matmuls
                # never target the same bank: the ~173ns PE->PSUM write
                # latency is hidden behind the sibling chains' compute
                qc, strips = st["qc"], st["strips"]
                pss = [spool.tile([128, 512], F32, name="ps") for _ in specs]
                for c in range(2):
                    for ps, (j, rhs_lo, w, _, _) in zip(pss, specs):
                        nc.tensor.matmul(
                            ps[:, 0:w],
                            lhsT=qc[c][:, j * 128:(j + 1) * 128],
                            rhs=qc[c][:, rhs_lo:rhs_lo + w],
                            start=(c == 0),
                            stop=(c == 1),
                        )
                for ps, (j, _, w, strip_off, masked) in zip(pss, specs):
                    dst = strips[j][:, strip_off:strip_off + w]
                    if masked:
                        mcopy(dst, ps[:, 0:w], mask512[:, 0:w])
                    else:
                        xcopy(dst, ps[:, 0:w])

            def mm1_group(st, G):
                g0 = G * 512
                # j-blocks strictly below the whole window: full 512 wide
                for j in range(0, 4 * G, 2):
                    mm1_chains(st, [
                        (j, g0, 512, g0 - 256 * (j // 2), False),
                        (j + 1, g0, 512, g0 - 256 * (j // 2), False),
                    ])
                # the 4 diagonal-containing j-blocks: each window starts
                # at its own diagonal block (widths 512/384/256/128), so
                # every masked copy uses mask512[:, :w]
                mm1_chains(st, [
                    (4 * G, g0, 512, 0, True),
                    (4 * G + 1, g0 + 128, 384, 128, True),
                    (4 * G + 2, g0 + 256, 256, 0, True),
                    (4 * G + 3, g0 + 384, 128, 128, True),
                ])

            def mm2_group(st, G, pool=None):
                g0 = G * 512
                vh, strips, ot_sb = st["vh"], st["strips"], st["ot"]
                pool = pool or opool
                otg = [
                    pool.tile([128, 512], F32, name=("ps" if pool is spool
                                                     else f"otg{c}"))
                    for c in range(2)
                ]
                last = 4 * G + 3
                for j in range(last + 1):
                    if j <= 4 * G - 1:
                        soff, w, ooff = g0 - 256 * (j // 2), 512, 0
                    elif j == 4 * G:
                        soff, w, ooff = 0, 512, 0
                    elif j == 4 * G + 1:
                        soff, w, ooff = 128, 384, 128
                    elif j == 4 * G + 2:
                        soff, w, ooff = 0, 256, 256
                    else:
                        soff, w, ooff = 128, 128, 384
                    for c in range(2):
                        nc.tensor.matmul(
                            otg[c][:, ooff:ooff + w],
                            lhsT=vh[:, j * n + c * 128:j * n + c * 128 + 128],
                            rhs=strips[j][:, soff:soff + w],
                            start=(j == 0),
                            stop=(j == last),
                        )
                # split drains across both copy engines to cut the
                # latency before the banks can be reused
                for c in range(2):
                    xcopy(ot_sb[c][:, g0:g0 + 256], otg[c][:, 0:256])
                    xcopy(ot_sb[c][:, g0 + 256:g0 + 512], otg[c][:, 256:512])

            def out_dma(st, lo, hi):
                # sync engine is idle once inputs are prefetched, so output
                # DMAs issue without queueing behind copy work
                for c in range(2):
                    psl = slice(c * 128, (c + 1) * 128)
                    nc.sync.dma_start(
                        out=o[st["h"]][psl, lo:hi], in_=st["ot"][c][:, lo:hi]
                    )

            # per-head schedule, software-pipelined across heads: strips of
            # group G are copied to SBUF while the PE runs the next group's
            # matmuls, and the next head's first mm1 group runs between
            # mm2(G2) and mm2(G3) so the otg PSUM banks have time to drain
            prev = None
            for h in range(h_per_core):
                st = alloc_head(h)
                mm1_group(st, 0)
                if prev is not None:
                    mm2_group(prev, 3)
                    out_dma(prev, 1536, t)
                mm1_group(st, 1)
                mm2_group(st, 0)
                mm1_group(st, 2)
                mm2_group(st, 1)
                out_dma(st, 0, 1024)
                mm1_group(st, 3)
                mm2_group(st, 2)
                out_dma(st, 1024, 1536)
                prev = st
            # final head's last group: mm1 is entirely done, so borrow two
            # spool banks instead of waiting on the otg banks' drain
            mm2_group(prev, 3, pool=spool)
            out_dma(prev, 1536, t)

    if waitsplit:
        _split_overloaded_waits(nc)
    return nc


_NC_CACHE = {}


def get_nc(h_per_core=H_PER_CORE, t=T, n=N):
    key = (h_per_core, t, n)
    if key not in _NC_CACHE:
        _NC_CACHE[key] = build_nc(h_per_core, t, n)
    return _NC_CACHE[key]


def make_in_maps(Q, V, n_cores=N_CORES):
    b, nh, t, n = Q.shape
    h_per_core = (b * nh) // n_cores
    q = np.asarray(Q, dtype=np.float64).reshape(b * nh, t, n)
    vf = np.asarray(V, dtype=np.float32).reshape(b * nh, t, n)
    # RoPE on host (elementwise preprocessing), fp64 for accuracy
    cos, sin_a = rope_tables(t, n, np.float64)
    qsw = np.empty_like(q)
    qsw[..., 0::2] = q[..., 1::2]
    qsw[..., 1::2] = q[..., 0::2]
    qr = q * cos + qsw * sin_a
    # pre-transposed (n, t) fp16 layout so the device needs only plain DMAs
    qrt = np.ascontiguousarray(qr.transpose(0, 2, 1)).astype(HF)
    # pre-arrange V into the device SBUF layout [p, t-tile, n] flattened to
    # [128, (t//128)*n] so the DMA per partition is one contiguous stream
    vb = np.ascontiguousarray(
        vf.astype(HF).reshape(b * nh, t // 128, 128, n).transpose(0, 2, 1, 3)
    ).reshape(b * nh, 128, (t // 128) * n)
    in_maps = []
    for c in range(n_cores):
        sl = slice(c * h_per_core, (c + 1) * h_per_core)
        in_maps.append(
            {
                "qrt": np.ascontiguousarray(qrt[sl]),
                "v": np.ascontiguousarray(vb[sl]),
            }
        )
    return in_maps


def assemble_output(res, b=B, nh=NH, t=T, n=N, n_cores=N_CORES):
    """Gather per-core O^T (h, n, t) fp16 outputs into (b, nh, t, n) fp32."""
    outs = [res.results[c]["o"] for c in range(n_cores)]
    ot = np.concatenate(outs, axis=0).astype(np.float32)  # (b*nh, n, t)
    return np.ascontiguousarray(ot.transpose(0, 2, 1)).reshape(b, nh, t, n)


def kernel(Q, K, V):
    """Full-input entry point: Q, K, V are (B, NH, T, N) float32 numpy arrays.
    K is unused (the module self-keys attention on rotated Q)."""
    Q = np.asarray(Q)
    V = np.asarray(V)
    b, nh, t, n = Q.shape
    nc = get_nc((b * nh) // N_CORES, t, n)
    in_maps = make_in_maps(Q, V, N_CORES)
    res = None
    last_err = None
    for attempt in range(3):  # retry transient device/runtime failures
        try:
            res = run_bass_kernel_spmd(
                nc, in_maps, core_ids=list(range(N_CORES)), trace=False
            )
            break
        except Exception as e:  # e.g. NRT_EXEC_UNIT_UNRECOVERABLE after a
            last_err = e  # wedged prior run; a clean retry usually recovers
            import time as _time

            _time.sleep(2.0 * (attempt + 1))
    if res is None:
        raise last_err
    return assemble_output(res, b, nh, t, n, N_CORES)
